# revision 1
# baseline (speedup 1.0000x reference)
"""CrossPSDLoss Trainium2 kernel.

Math (from the reference):
  res = target - pred; both [1024, 16384] f32.
  cross rows i=0..15: row i = concat_b x[b, 1024*i : 1024*(i+1)]  (length 1048576)
  Welch per row: 511 frames of 4096 (stride 2048), periodic-hann*2 window,
  rFFT, power, sum over frames -> S[k].  Loss only uses rows 8..15 and
  frequency bins 21..499 (the (20,500) mask with df=1), and the /T factors
  cancel in the ratio:
     out = (2/480) * sum_{row=8..15} sum_{kb=21..499} S_res[row,kb]/S_tgt[row,kb]

Sharding: one Welch row per NeuronCore (8 rows, 8 cores); each core consumes
only its [1024, 1024] column slice of pred/target.  No collectives; the host
sums the 8 per-core partial scalars.

Per-core pipeline:
  - host pre-casts the slice to bf16 (verified: final rel err ~1e-5)
  - DMA-transpose load -> XT[p, 1024*t + b] = X[b, 128*t + p]  (samples on
    partitions, which the TensorE contraction requires);
    frame_f[k] = XT[p, 1024*t + 2f + q] for k = 1024*q + 128*t + p = 128*j + p
  - res = tgt - pred on DVE (bf16)
  - even/odd fold (win/cos symmetric, sin antisymmetric about k=4096-k):
      u[k] = x[k] + x[4096-k],  v[k] = x[k] - x[4096-k],  k = 0..2047
      Re[n,f] = sum_{k=0..2047} C[k,n] u[k,f] + C[2048,n] x[2048,f]
      Im[n,f] = sum_{k=0..2047} S[k,n] v[k,f]
    built per 128-k-tile j=0..15 as psB = J0 @ B_j (+ row-0 partner
    mini-matmul), U_j = A_j + psB, V_j = A_j - psB on DVE, where
    A_j = y_j, B_j = y_{31-j}, J0 = anti-identity with row 0 zeroed.
    This HALVES the DFT GEMM contraction (16 k-tiles instead of 32).
  - windowed DFT GEMMs vs precomputed folded weights (bins 21..499 only),
    psum [chunk<=120, 511 frames]
  - PSD: Square activation with accum over frames, ratio + reduce on device.
"""

import os
import sys
from contextlib import ExitStack

import numpy as np
import ml_dtypes

for _p in ("/opt/trn_rl_repo", "/root/.axon_site/_ro/trn_rl_repo"):
    if os.path.isdir(_p) and _p not in sys.path:
        sys.path.insert(0, _p)

import concourse.bass as bass
import concourse.mybir as mybir
from concourse import bacc, tile
from concourse.bass_utils import run_bass_kernel_spmd

BF16 = ml_dtypes.bfloat16

NPERSEG = 4096
NSEG = 511
NBINS = 479          # bins 21..499
CHUNKS = [120, 120, 120, 119]   # 479 split into 4 partition chunks
N_CORES = 8
ROW0 = 8             # first Welch row that matters


def _y_ap(xtile, m):
    """AP of y_m[p, f] = frame_f[128*m + p] = XT[p, 1024*t + 2f + q],
    m = 8*q + t, for all 128 partitions and f = 0..510."""
    q, t = divmod(m, 8)
    base = 1024 * t + q
    return xtile[:, base: base + 1021: 2]


def _y0_ap(xtile, m):
    """Row-0 slice of _y_ap(xtile, m); also valid for m == 32 (q=4, t=0),
    whose weight row is zero."""
    q, t = divmod(m, 8)
    base = 1024 * t + q
    return xtile[0:1, base: base + 1021: 2]


def _build_nc() -> bass.Bass:
    # Bacc (not bass.Bass): its compile() runs generate_event_semaphores(),
    # which splits multi-semaphore waits into event-sem chains — TRN2
    # instructions support at most one wait each.
    nc = bacc.Bacc("TRN2", target_bir_lowering=False, debug=False,
                   num_devices=N_CORES)
    dt = mybir.dt

    # x inputs arrive t-major ([t, b, p] with p = column-within-128-block) so
    # every DMA-transpose reads a fully contiguous source (~350 GB/s instead
    # of the ~261 GB/s non-contiguous-mid-dim rate).
    xp_d = nc.dram_tensor("xp", [8, 1024, 128], dt.bfloat16,
                          kind="ExternalInput")
    xt_d = nc.dram_tensor("xt", [8, 1024, 128], dt.bfloat16,
                          kind="ExternalInput")
    wu_d = nc.dram_tensor("wu", [128, 16, NBINS], dt.bfloat16,
                          kind="ExternalInput")
    wv_d = nc.dram_tensor("wv", [128, 16, NBINS], dt.bfloat16,
                          kind="ExternalInput")
    wj0_d = nc.dram_tensor("wj0", [128, 128], dt.bfloat16,
                           kind="ExternalInput")
    w2k_d = nc.dram_tensor("w2k", [1, NBINS], dt.bfloat16,
                           kind="ExternalInput")
    out_d = nc.dram_tensor("out", [1, 1], dt.float32, kind="ExternalOutput")

    with ExitStack() as ctx:
        tc = ctx.enter_context(tile.TileContext(nc))
        xpool = ctx.enter_context(tc.tile_pool(name="x", bufs=1))
        wpool = ctx.enter_context(tc.tile_pool(name="w", bufs=1))
        uvpool = ctx.enter_context(tc.tile_pool(name="uv", bufs=1))
        psb = ctx.enter_context(tc.tile_pool(name="psb", bufs=4, space="PSUM"))
        pspool = ctx.enter_context(tc.tile_pool(name="ps", bufs=3, space="PSUM"))
        ps1 = ctx.enter_context(tc.tile_pool(name="ps1", bufs=1, space="PSUM"))
        scpool = ctx.enter_context(tc.tile_pool(name="sc", bufs=4))
        stat = ctx.enter_context(tc.tile_pool(name="stat", bufs=1))

        wu_sb = wpool.tile([128, 16, NBINS], dt.bfloat16, tag="wu")
        wv_sb = wpool.tile([128, 16, NBINS], dt.bfloat16, tag="wv")
        j0_sb = wpool.tile([128, 128], dt.bfloat16, tag="wj0")
        w2k_sb = wpool.tile([1, NBINS], dt.bfloat16, tag="w2k")
        xt_t = xpool.tile([128, 8192], dt.bfloat16, tag="xt_t")
        xp_t = xpool.tile([128, 8192], dt.bfloat16, tag="xp_t")
        xr_t = xpool.tile([128, 8192], dt.bfloat16, tag="xr_t")

        # DMA order = PE need order: xt tiles + J0 unblock the fold phase of
        # the tgt input first, then the GEMM weights, then xp for res.
        nc.sync.dma_start(j0_sb[:, :], wj0_d[:, :])
        nc.sync.dma_start(w2k_sb[:, :], w2k_d[:, :])
        for t in range(8):
            sl = slice(1024 * t, 1024 * (t + 1))
            nc.sync.dma_start(xt_t[:, sl], xt_d[t], transpose=True)
        nc.sync.dma_start(wu_sb[:, :, :], wu_d[:, :, :])
        nc.sync.dma_start(wv_sb[:, :, :], wv_d[:, :, :])
        for t in range(8):
            sl = slice(1024 * t, 1024 * (t + 1))
            nc.sync.dma_start(xp_t[:, sl], xp_d[t], transpose=True)
        for t in range(8):
            sl = slice(1024 * t, 1024 * (t + 1))
            nc.vector.tensor_sub(xr_t[:, sl], xt_t[:, sl], xp_t[:, sl])

        RATIO = stat.tile([128, 4], dt.float32)
        nc.vector.memset(RATIO[:, :], 0.0)
        ones = stat.tile([128, 1], dt.float32)
        nc.vector.memset(ones[:, :], 1.0)
        # e0: [1, 128] unit row vector; e0.T @ y0 writes y0 into psum row 0
        # and zeros rows 1..127 (full-region group open for the J0 matmul).
        e0 = stat.tile([1, 128], dt.bfloat16)
        nc.vector.memset(e0[:, :], 0.0)
        nc.vector.memset(e0[0:1, 0:1], 1.0)

        # Fold (both inputs first, so the PE's J0 matmuls for input 2 hide
        # the DVE U/V builds of input 1):
        #   psB_j = J0 @ y_{31-j}  (+ row-0 partner y_{32-j}[0]),
        #   U_j = y_j + psB_j, V_j = y_j - psB_j  (bf16, on DVE).
        UV = {}
        for xi, xtile in ((1, xt_t), (0, xr_t)):
            U = []
            V = []
            for j in range(16):
                pb = psb.tile([128, NSEG], dt.float32, tag="psB")
                # Row-0 partner first (e0.T @ y0 — full-region, opens the
                # group), then the J0 matmul closes it: J0's row 0 is
                # all-zero, so it accumulates 0 onto the partner row.
                nc.tensor.matmul(pb[:, :], e0[:, :],
                                 _y0_ap(xtile, 32 - j),
                                 start=True, stop=False)
                nc.tensor.matmul(pb[:, :], j0_sb[:, :], _y_ap(xtile, 31 - j),
                                 start=False, stop=True)
                u = uvpool.tile([128, NSEG], dt.bfloat16, tag=f"U{xi}_{j}")
                v = uvpool.tile([128, NSEG], dt.bfloat16, tag=f"V{xi}_{j}")
                # Bounce psB to SBUF bf16 on ACT so the DVE add/sub run in
                # 2x bf16 mode instead of 1x against fp32 PSUM.
                pbs = scpool.tile([128, NSEG], dt.bfloat16, tag="pbs")
                nc.scalar.copy(pbs[:, :], pb[:, :])
                nc.vector.tensor_add(u[:, :], _y_ap(xtile, j), pbs[:, :])
                nc.vector.tensor_sub(v[:, :], _y_ap(xtile, j), pbs[:, :])
                U.append(u)
                V.append(v)
            UV[xi] = (U, V)

        # E[(xi, trig, c)]: per-bin sum over the 511 frames of out^2 for
        # chunk c of the {cos,sin} DFT of input xi (0=res, 1=tgt).
        E = {}
        for xi, xtile in ((1, xt_t), (0, xr_t)):
            U, V = UV[xi]
            for m in range(8):
                c = m % 4
                trig = m // 4
                rows = CHUNKS[c]
                col0 = 120 * c
                w_sb = wu_sb if trig == 0 else wv_sb
                tiles = U if trig == 0 else V
                ps = pspool.tile([128, NSEG], dt.float32, tag="gemm_ps")
                for j in range(16):
                    nc.tensor.matmul(
                        ps[:rows, :],
                        w_sb[:, j, col0:col0 + rows],
                        tiles[j][:, :],
                        start=(j == 0),
                        stop=(trig == 1 and j == 15),
                    )
                if trig == 0:
                    # k = 2048 singleton (sin weight there is 0)
                    nc.tensor.matmul(
                        ps[:rows, :],
                        w2k_sb[:, col0:col0 + rows],
                        _y0_ap(xtile, 16),
                        start=False, stop=True)
                tmp = scpool.tile([128, NSEG], dt.float32, tag="sq")
                acc = stat.tile([128, 1], dt.float32, tag=f"E{xi}_{m}")
                E[(xi, trig, c)] = acc
                nc.scalar.activation(
                    out=tmp[:rows, :],
                    in_=ps[:rows, :],
                    func=mybir.ActivationFunctionType.Square,
                    accum_out=acc[:rows, :],
                )

        for c in range(4):
            rows = CHUNKS[c]
            sr = stat.tile([128, 1], dt.float32, tag=f"SR{c}")
            st = stat.tile([128, 1], dt.float32, tag=f"ST{c}")
            rec = stat.tile([128, 1], dt.float32, tag=f"REC{c}")
            nc.vector.tensor_add(sr[:rows, :], E[(0, 0, c)][:rows, :],
                                 E[(0, 1, c)][:rows, :])
            nc.vector.tensor_add(st[:rows, :], E[(1, 0, c)][:rows, :],
                                 E[(1, 1, c)][:rows, :])
            nc.vector.reciprocal(rec[:rows, :], st[:rows, :])
            nc.vector.tensor_mul(RATIO[:rows, c:c + 1], sr[:rows, :],
                                 rec[:rows, :])

        tot = ps1.tile([1, 4], dt.float32)
        nc.tensor.matmul(tot[:1, :4], ones[:, :1], RATIO[:, :4],
                         start=True, stop=True)
        scaled = stat.tile([1, 4], dt.float32)
        nc.vector.tensor_scalar_mul(scaled[:1, :], tot[:1, :], 2.0 / 480.0)
        red = stat.tile([1, 1], dt.float32)
        nc.vector.tensor_reduce(red[:1, :1], scaled[:1, :],
                                axis=mybir.AxisListType.X,
                                op=mybir.AluOpType.add)
        nc.sync.dma_start(out_d[:, :], red[:1, :1])

    nc.compile()
    return nc


def _build_w():
    """Folded DFT weights, all bf16:
      wu[p, j, n] = win[k] cos(2 pi k kb_n / 4096), k = 128 j + p  (u weights)
      wv[p, j, n] = win[k] sin(...)                                (v weights)
      wj0 = anti-identity J0[p, 128-p] = 1 for p = 1..127, row 0 zero
      w2k[0, n]  = win[2048] cos(2 pi 2048 kb_n / 4096)
    """
    k = np.arange(NPERSEG, dtype=np.float64)
    win = (0.5 - 0.5 * np.cos(2.0 * np.pi * k / NPERSEG)) * 2.0
    kb = np.arange(21, 21 + NBINS, dtype=np.float64)
    ang = 2.0 * np.pi * np.outer(k, kb) / NPERSEG
    C = win[:, None] * np.cos(ang)
    S = win[:, None] * np.sin(ang)
    wu = np.ascontiguousarray(
        C[:2048].reshape(16, 128, NBINS).transpose(1, 0, 2)).astype(BF16)
    wv = np.ascontiguousarray(
        S[:2048].reshape(16, 128, NBINS).transpose(1, 0, 2)).astype(BF16)
    j0 = np.zeros((128, 128), np.float64)
    for p in range(1, 128):
        j0[p, 128 - p] = 1.0
    w2k = np.ascontiguousarray(C[2048:2049]).astype(BF16)
    return {
        "wu": wu,
        "wv": wv,
        "wj0": j0.astype(BF16),
        "w2k": w2k,
    }


_CACHE: dict = {}


def _get_prog():
    if "nc" not in _CACHE:
        _CACHE["nc"] = _build_nc()
    return _CACHE["nc"]


def _get_w():
    if "w" not in _CACHE:
        _CACHE["w"] = _build_w()
    return _CACHE["w"]


def kernel(pred: np.ndarray, target: np.ndarray, _trace: bool = False):
    nc = _get_prog()
    w = _get_w()
    pred = np.asarray(pred)
    target = np.asarray(target)
    in_maps = []
    for i in range(N_CORES):
        c0 = (ROW0 + i) * 1024
        in_maps.append({
            "xp": np.ascontiguousarray(
                pred[:, c0:c0 + 1024].astype(BF16)
                .reshape(1024, 8, 128).transpose(1, 0, 2)),
            "xt": np.ascontiguousarray(
                target[:, c0:c0 + 1024].astype(BF16)
                .reshape(1024, 8, 128).transpose(1, 0, 2)),
            **w,
        })
    res = run_bass_kernel_spmd(nc, in_maps, list(range(N_CORES)), trace=_trace)
    total = float(sum(float(res.results[i]["out"][0, 0])
                      for i in range(N_CORES)))
    out = np.array(total, dtype=np.float32)
    if _trace:
        return out, res
    return out



# revision 5
# speedup vs baseline: 2.1258x; 2.1258x over previous
"""CrossPSDLoss Trainium2 kernel (fp8 DoubleRow version).

Math (from the reference):
  res = target - pred; both [1024, 16384] f32.
  cross rows i=0..15: row i = concat_b x[b, 1024*i : 1024*(i+1)]  (length 1048576)
  Welch per row: 511 frames of 4096 (stride 2048), periodic-hann*2 window,
  rFFT, power, sum over frames -> S[k].  Loss only uses rows 8..15 and
  frequency bins 21..499 (the (20,500) mask with df=1), and the /T factors
  cancel in the ratio:
     out = (2/480) * sum_{row=8..15} sum_{kb=21..499} S_res[row,kb]/S_tgt[row,kb]

Sharding: one Welch row per NeuronCore (8 rows, 8 cores); each core consumes
only its [1024, 1024] column slice of pred/target.  No collectives; the host
sums the 8 per-core partial scalars.

Per-core pipeline (cost-model driven):
  - host computes res = target - pred in f32, casts res/target to fp8e4m3,
    and pre-transposes to XT[p, t, b] = X[b, 128*t + p] so every device DMA
    is a plain contiguous copy (1KB+ runs -> full modeled DMA rate).
  - frame_f[k] for k = 128*m + p lives at XT[p, m%8, (m//8) + 2f], so the
    windowed-DFT GEMM's moving operand is a strided AP of the X tile; no
    on-device data movement at all.
  - DFT-as-GEMM in fp8e4 with perf_mode=DoubleRow: each matmul contracts
    K=256 (two 128-row k-tiles via the [128, 2, n] AP form) at 0.5
    cycles/row in the cost model -- 4x less PE time than bf16 K=128
    matmuls.  Full 4096-tap window folded into the weights (no even/odd
    fold: at DoubleRow rates the fold's J0/e0 matmuls cost exactly what
    they save).  fp8 quantization error cancels almost entirely in the
    psd_res/psd_target ratio (verified: ~1e-5 end-to-end).
  - PSD: Square activation with accum over frames; ratio + reduce on device.
"""

import os
import sys
from contextlib import ExitStack

import numpy as np
import ml_dtypes

for _p in ("/opt/trn_rl_repo", "/root/.axon_site/_ro/trn_rl_repo"):
    if os.path.isdir(_p) and _p not in sys.path:
        sys.path.insert(0, _p)

import concourse.bass as bass
import concourse.mybir as mybir
from concourse import bacc, tile
from concourse.bass_utils import run_bass_kernel_spmd

FP8 = ml_dtypes.float8_e4m3

NPERSEG = 4096
NSEG = 511
NBINS = 479          # bins 21..499
CHUNKS = [120, 120, 120, 119]   # 479 split into 4 partition chunks
N_CORES = 8
ROW0 = 8             # first Welch row that matters


def _build_nc() -> bass.Bass:
    # Bacc (not bass.Bass): its compile() runs generate_event_semaphores(),
    # which splits multi-semaphore waits into event-sem chains — TRN2
    # instructions support at most one wait each.
    nc = bacc.Bacc("TRN2", target_bir_lowering=False, debug=False,
                   num_devices=N_CORES)
    dt = mybir.dt
    DR = mybir.MatmulPerfMode.DoubleRow

    xr_d = nc.dram_tensor("xr", [128, 8, 1024], dt.float8e4,
                          kind="ExternalInput")
    xt_d = nc.dram_tensor("xt", [128, 8, 1024], dt.float8e4,
                          kind="ExternalInput")
    # weights chunk-major [chunk, p, m, 128]: cols 0..119 are bins
    # 21+120c..., cols >= CHUNKS[c] are zero padding (never read).
    wc_d = nc.dram_tensor("wc", [4, 128, 32, 128], dt.float8e4,
                          kind="ExternalInput")
    ws_d = nc.dram_tensor("ws", [4, 128, 32, 128], dt.float8e4,
                          kind="ExternalInput")
    out_d = nc.dram_tensor("out", [1, 1], dt.float32, kind="ExternalOutput")

    with ExitStack() as ctx:
        tc = ctx.enter_context(tile.TileContext(nc))
        xpool = ctx.enter_context(tc.tile_pool(name="x", bufs=1))
        wpool = ctx.enter_context(tc.tile_pool(name="w", bufs=1))
        pspool = ctx.enter_context(tc.tile_pool(name="ps", bufs=4,
                                                space="PSUM"))
        ps1 = ctx.enter_context(tc.tile_pool(name="ps1", bufs=1, space="PSUM"))
        scpool = ctx.enter_context(tc.tile_pool(name="sc", bufs=2))
        stat = ctx.enter_context(tc.tile_pool(name="stat", bufs=1))

        xr_t = xpool.tile([128, 8, 1024], dt.float8e4, tag="xr")
        xt_t = xpool.tile([128, 8, 1024], dt.float8e4, tag="xt")
        w_sb = {}
        for trig in range(2):
            for c in range(4):
                w_sb[(trig, c)] = wpool.tile([128, 32, 128], dt.float8e4,
                                             name=f"w{trig}_{c}",
                                             tag=f"w{trig}_{c}")

        # E accumulators, column c = chunk c.  Partitions with no real bin
        # keep their memset value: res-E 0.0 / tgt-E 1.0 makes their ratio
        # an exact 0/2 = 0, so no masking is needed before the reduction.
        E = {}
        for xi in range(2):
            for trig in range(2):
                t_ = stat.tile([128, 4], dt.float32, name=f"E{xi}_{trig}",
                               tag=f"E{xi}_{trig}")
                nc.vector.memset(t_[:, :], 0.0 if xi == 0 else 1.0)
                E[(xi, trig)] = t_
        ones = stat.tile([128, 1], dt.float32, tag="ones")
        nc.vector.memset(ones[:, :], 1.0)

        # DMA order = PE need order.  First cos-chunk0 weights (in 4 pieces
        # so the first matmuls start after ~1us) interleaved with the res
        # t-blocks, then tgt, then the remaining weight chunks.
        for j in range(4):
            nc.sync.dma_start(w_sb[(0, 0)][:, 8 * j:8 * j + 8, :],
                              wc_d[0][:, 8 * j:8 * j + 8, :])
            nc.sync.dma_start(xr_t[:, 2 * j, :], xr_d[:, 2 * j, :])
            nc.sync.dma_start(xr_t[:, 2 * j + 1, :], xr_d[:, 2 * j + 1, :])
        for t in range(8):
            nc.sync.dma_start(xt_t[:, t, :], xt_d[:, t, :])
        for c in range(1, 4):
            nc.sync.dma_start(w_sb[(0, c)][:, :, :], wc_d[c])
        for c in range(4):
            nc.sync.dma_start(w_sb[(1, c)][:, :, :], ws_d[c])

        # Windowed-DFT GEMMs + per-bin power accumulation.
        for trig in range(2):
            for c in range(4):
                rows = CHUNKS[c]
                for xi, xtile in ((0, xr_t), (1, xt_t)):
                    ps = pspool.tile([128, NSEG], dt.float32, tag="gemm_ps")
                    for i in range(16):
                        m = 2 * i
                        a, t = divmod(m, 8)
                        nc.tensor.matmul(
                            ps[:rows, :],
                            w_sb[(trig, c)][:, m:m + 2, 0:rows],
                            xtile[:, t:t + 2, a:a + 1021:2],
                            start=(i == 0),
                            stop=(i == 15),
                            perf_mode=DR,
                        )
                    tmp = scpool.tile([128, NSEG], dt.float32, tag="sq")
                    nc.scalar.activation(
                        out=tmp[:rows, :],
                        in_=ps[:rows, :],
                        func=mybir.ActivationFunctionType.Square,
                        accum_out=E[(xi, trig)][:rows, c:c + 1],
                    )

        # ratio = (res_cos^2+res_sin^2) / (tgt_cos^2+tgt_sin^2), 4-wide.
        SR = stat.tile([128, 4], dt.float32, tag="SR")
        ST = stat.tile([128, 4], dt.float32, tag="ST")
        REC = stat.tile([128, 4], dt.float32, tag="REC")
        RATIO = stat.tile([128, 4], dt.float32, tag="RATIO")
        nc.vector.tensor_add(SR[:, :], E[(0, 0)][:, :], E[(0, 1)][:, :])
        nc.vector.tensor_add(ST[:, :], E[(1, 0)][:, :], E[(1, 1)][:, :])
        nc.vector.reciprocal(REC[:, :], ST[:, :])
        nc.vector.tensor_mul(RATIO[:, :], SR[:, :], REC[:, :])

        tot = ps1.tile([1, 4], dt.float32)
        nc.tensor.matmul(tot[:1, :4], ones[:, :1], RATIO[:, :4],
                         start=True, stop=True)
        scaled = stat.tile([1, 4], dt.float32, tag="scaled")
        nc.vector.tensor_scalar_mul(scaled[:1, :], tot[:1, :], 2.0 / 480.0)
        red = stat.tile([1, 1], dt.float32, tag="red")
        nc.vector.tensor_reduce(red[:1, :1], scaled[:1, :],
                                axis=mybir.AxisListType.X,
                                op=mybir.AluOpType.add)
        nc.sync.dma_start(out_d[:, :], red[:1, :1])

    nc.compile()
    return nc


def _build_w():
    """Windowed DFT weights, fp8e4m3, chunk-major with zero padding:
      wc[c, p, m, j] = win[k] cos(2 pi k (21+120c+j) / 4096), k = 128 m + p
      ws[c, p, m, j] = win[k] sin(...)
    """
    k = np.arange(NPERSEG, dtype=np.float64)
    win = (0.5 - 0.5 * np.cos(2.0 * np.pi * k / NPERSEG)) * 2.0
    kb = np.arange(21, 21 + NBINS, dtype=np.float64)
    ang = 2.0 * np.pi * np.outer(k, kb) / NPERSEG
    C = win[:, None] * np.cos(ang)
    S = win[:, None] * np.sin(ang)

    def pack(W):
        Wp = W.reshape(32, 128, NBINS).transpose(1, 0, 2)  # [p, m, bins]
        out = np.zeros((4, 128, 32, 128), np.float64)
        for c in range(4):
            rows = CHUNKS[c]
            out[c, :, :, 0:rows] = Wp[:, :, 120 * c:120 * c + rows]
        return out.astype(FP8)

    return {"wc": pack(C), "ws": pack(S)}


_CACHE: dict = {}


def _get_prog():
    if "nc" not in _CACHE:
        _CACHE["nc"] = _build_nc()
    return _CACHE["nc"]


def _get_w():
    if "w" not in _CACHE:
        _CACHE["w"] = _build_w()
    return _CACHE["w"]


def kernel(pred: np.ndarray, target: np.ndarray, _trace: bool = False):
    nc = _get_prog()
    w = _get_w()
    pred = np.asarray(pred)
    target = np.asarray(target)
    # only Welch rows 8..15 contribute -> only the right half of the columns
    tgt_half = np.ascontiguousarray(target[:, 8192:]).astype(np.float32)
    res_half = tgt_half - pred[:, 8192:]
    res8 = res_half.astype(FP8)
    tgt8 = tgt_half.astype(FP8)
    in_maps = []
    for i in range(N_CORES):
        sl = slice(1024 * i, 1024 * (i + 1))
        in_maps.append({
            "xr": np.ascontiguousarray(
                res8[:, sl].reshape(1024, 8, 128).transpose(2, 1, 0)),
            "xt": np.ascontiguousarray(
                tgt8[:, sl].reshape(1024, 8, 128).transpose(2, 1, 0)),
            **w,
        })
    res = run_bass_kernel_spmd(nc, in_maps, list(range(N_CORES)), trace=_trace)
    total = float(sum(float(res.results[i]["out"][0, 0])
                      for i in range(N_CORES)))
    out = np.array(total, dtype=np.float32)
    if _trace:
        return out, res
    return out


# revision 7
# speedup vs baseline: 3.4507x; 1.6232x over previous
"""CrossPSDLoss Trainium2 kernel (half-frame fp8 DoubleRow version).

Math (from the reference):
  res = target - pred; both [1024, 16384] f32.
  cross rows i=0..15: row i = concat_b x[b, 1024*i : 1024*(i+1)]  (length 1048576)
  Welch per row: 511 frames of 4096 (stride 2048), periodic-hann*2 window,
  rFFT, power, sum over frames -> S[n].  Loss only uses rows 8..15 and
  frequency bins 21..499, and the /T factors cancel in the ratio:
     out = (2/480) * sum_{row=8..15} sum_{n=21..499} S_res[row,n]/S_tgt[row,n]

Half-frame trick: the scaled periodic hann is win[k] = 1 - cos(theta k)
(theta = 2pi/4096), and win[k+2048] = 1 + cos(theta k).  With the
unwindowed half-frame DFT R_h[n] = sum_{k<2048} x[2048h+k] e^{-i theta n k}:
     F_f[n] = (R_f[n] - C_f[n]) + (-1)^n (R_{f+1}[n] + C_{f+1}[n]),
     C_h[n] = (R_h[n-1] + R_h[n+1]) / 2.
Each sample enters exactly one GEMM per (trig, bin-chunk) — the 50% frame
overlap is never recomputed, halving the main GEMM work.

Sharding: one Welch row per NeuronCore (8 rows, 8 cores); host sums the 8
per-core scalars.  Host computes res, scales by 1/4 (keeps R in fp8e4m3
range; the ratio is scale-invariant), casts to fp8, and pre-transposes to
XT[p, t, b] = X[b, 128t + p] so all DMAs are contiguous.

Per-core pipeline, per (input, trig, chunk-of-122-bins-with-halo) group:
  1. PE: 8 fp8 DoubleRow matmuls (K=256 each, 0.5 cycles/row in the cost
     model) -> psR[nin, 512] = R at the chunk's halo bin range x 512 halves.
  2. DVE: copy psR -> Rb (fp8, SBUF).
  3. PE: ONE DoubleRow matmul with a hand-built overlapping moving AP
     ([nin, 2@stride1, 511]) and the stacked tridiagonal weights
     T1 = tridiag(-1/2, 1, -1/2), T2 = diag((-1)^n) tridiag(1/2, 1, 1/2):
     psF = T1 @ Rb[:, 0:511] + T2 @ Rb[:, 1:512]  (both halves of F in one
     instruction, accumulated in PSUM; verified bit-exact on HW).
  4. ACT: Square activation with accum over the 511 frames -> E column.
PE emission is software-pipelined (group g's tridiag is emitted after
group g+1's main matmuls) so the PE never waits for the DVE bounce.
"""

import os
import sys
from contextlib import ExitStack

import numpy as np
import ml_dtypes

for _p in ("/opt/trn_rl_repo", "/root/.axon_site/_ro/trn_rl_repo"):
    if os.path.isdir(_p) and _p not in sys.path:
        sys.path.insert(0, _p)

import concourse.bass as bass
import concourse.mybir as mybir
from concourse import bacc, tile
from concourse.bass_utils import run_bass_kernel_spmd

FP8 = ml_dtypes.float8_e4m3

NSEG = 511
NH = 512             # half-frames
NBINS = 479          # bins 21..499
B0S = [21, 141, 261, 381]
CHUNKS = [120, 120, 120, 119]
N_CORES = 8
ROW0 = 8
XSCALE = 0.25


def _overlap_ap(ap2d):
    """[P, 512] 2D slice -> [P, 2, 511] AP, dim1 stride 1 (overlapping):
    element (p, r, f) reads column f + r."""
    y = ap2d.unsqueeze(1).copy()
    v = y.ap
    v.pop(1)
    v.insert(1, (1, 2))
    return y


def _build_nc() -> bass.Bass:
    nc = bacc.Bacc("TRN2", target_bir_lowering=False, debug=False,
                   num_devices=N_CORES)
    dt = mybir.dt
    DR = mybir.MatmulPerfMode.DoubleRow

    xr_d = nc.dram_tensor("xr", [128, 8, 1024], dt.float8e4,
                          kind="ExternalInput")
    xt_d = nc.dram_tensor("xt", [128, 8, 1024], dt.float8e4,
                          kind="ExternalInput")
    # half-DFT weights, chunk-major [chunk, p, m, 128]; cols j < nin are
    # bins b0-1+j (halo included), the rest zero padding (never read).
    wc_d = nc.dram_tensor("wc", [4, 128, 16, 128], dt.float8e4,
                          kind="ExternalInput")
    ws_d = nc.dram_tensor("ws", [4, 128, 16, 128], dt.float8e4,
                          kind="ExternalInput")
    t_d = nc.dram_tensor("tmat", [128, 2, 128], dt.float8e4,
                         kind="ExternalInput")
    out_d = nc.dram_tensor("out", [1, 1], dt.float32, kind="ExternalOutput")

    with ExitStack() as ctx:
        tc = ctx.enter_context(tile.TileContext(nc))
        xpool = ctx.enter_context(tc.tile_pool(name="x", bufs=1))
        wpool = ctx.enter_context(tc.tile_pool(name="w", bufs=1))
        psR = ctx.enter_context(tc.tile_pool(name="psR", bufs=3,
                                             space="PSUM"))
        psF = ctx.enter_context(tc.tile_pool(name="psF", bufs=3,
                                             space="PSUM"))
        ps1 = ctx.enter_context(tc.tile_pool(name="ps1", bufs=1, space="PSUM"))
        rbpool = ctx.enter_context(tc.tile_pool(name="rb", bufs=3))
        scpool = ctx.enter_context(tc.tile_pool(name="sc", bufs=2))
        stat = ctx.enter_context(tc.tile_pool(name="stat", bufs=1))

        xr_t = xpool.tile([128, 8, 1024], dt.float8e4, tag="xr")
        xt_t = xpool.tile([128, 8, 1024], dt.float8e4, tag="xt")
        t_t = wpool.tile([128, 2, 128], dt.float8e4, tag="tmat")
        w_sb = {}
        for trig in range(2):
            for c in range(4):
                w_sb[(trig, c)] = wpool.tile([128, 16, 128], dt.float8e4,
                                             name=f"w{trig}_{c}",
                                             tag=f"w{trig}_{c}")

        # E accumulators, column c = chunk c.  Partitions with no real bin
        # keep their memset value: res-E 0.0 / tgt-E 1.0 makes their ratio
        # an exact 0/2 = 0, so no masking is needed before the reduction.
        E = {}
        for xi in range(2):
            for trig in range(2):
                t_ = stat.tile([128, 4], dt.float32, name=f"E{xi}_{trig}",
                               tag=f"E{xi}_{trig}")
                nc.vector.memset(t_[:, :], 0.0 if xi == 0 else 1.0)
                E[(xi, trig)] = t_
        # the 2/480 loss scale rides on the reduction weights
        ones = stat.tile([128, 1], dt.float32, tag="ones")
        nc.vector.memset(ones[:, :], 2.0 / 480.0)

        # DMA order = PE need order (res groups run before tgt groups).
        nc.sync.dma_start(t_t[:, :, :], t_d[:, :, :])
        nc.sync.dma_start(w_sb[(0, 0)][:, :, :], wc_d[0])
        nc.sync.dma_start(xr_t[:, 0:4, :], xr_d[:, 0:4, :])
        nc.sync.dma_start(xr_t[:, 4:8, :], xr_d[:, 4:8, :])
        for c in range(1, 4):
            nc.sync.dma_start(w_sb[(0, c)][:, :, :], wc_d[c])
        for c in range(4):
            nc.sync.dma_start(w_sb[(1, c)][:, :, :], ws_d[c])
        nc.sync.dma_start(xt_t[:, 0:4, :], xt_d[:, 0:4, :])
        nc.sync.dma_start(xt_t[:, 4:8, :], xt_d[:, 4:8, :])

        # one group = (input, trig, chunk); res first so xt can arrive late.
        groups = [(xi, trig, c)
                  for xi in range(2) for trig in range(2) for c in range(4)]

        def emit_main(g):
            xi, trig, c = g
            xtile = xr_t if xi == 0 else xt_t
            nin = CHUNKS[c] + 2
            ps = psR.tile([128, NH], dt.float32, tag="psR")
            # pair order: (a=0,t=0..3), (a=1,t=0..3), (a=0,t=4..7), ... so
            # the first 4 matmuls only need the first half of the x tile.
            order = [0, 1, 4, 5, 2, 3, 6, 7]
            for n_, i in enumerate(order):
                m = 2 * i
                a, t = divmod(m, 8)
                nc.tensor.matmul(
                    ps[:nin, :],
                    w_sb[(trig, c)][:, m:m + 2, 0:nin],
                    xtile[:, t:t + 2, a:a + 1023:2],
                    start=(n_ == 0),
                    stop=(n_ == 7),
                    perf_mode=DR,
                )
            return ps

        def emit_bounce(g, ps):
            xi, trig, c = g
            nin = CHUNKS[c] + 2
            rb = rbpool.tile([128, NH], dt.float8e4, tag="rb")
            nc.vector.tensor_copy(rb[:nin, :], ps[:nin, :])
            return rb

        def emit_tail(g, rb):
            xi, trig, c = g
            rows = CHUNKS[c]
            nin = rows + 2
            pf = psF.tile([128, NSEG], dt.float32, tag="psF")
            nc.tensor.matmul(
                pf[:rows, :],
                t_t[0:nin, :, 0:rows],
                _overlap_ap(rb[0:nin, 0:NSEG]),
                start=True, stop=True,
                perf_mode=DR,
            )
            tmp = scpool.tile([128, NSEG], dt.float32, tag="sq")
            nc.scalar.activation(
                out=tmp[:rows, :],
                in_=pf[:rows, :],
                func=mybir.ActivationFunctionType.Square,
                accum_out=E[(xi, trig)][:rows, c:c + 1],
            )

        # software pipeline: group g's tridiag+square are emitted after
        # group g+1's main matmuls, so the PE never waits on the DVE copy.
        pend = None
        for g in groups:
            ps = emit_main(g)
            rb = emit_bounce(g, ps)
            if pend is not None:
                emit_tail(*pend)
            pend = (g, rb)
        emit_tail(*pend)

        # ratio = (res_cos^2+res_sin^2) / (tgt_cos^2+tgt_sin^2), 4-wide.
        SR = stat.tile([128, 4], dt.float32, tag="SR")
        ST = stat.tile([128, 4], dt.float32, tag="ST")
        REC = stat.tile([128, 4], dt.float32, tag="REC")
        RATIO = stat.tile([128, 4], dt.float32, tag="RATIO")
        nc.vector.tensor_add(SR[:, :], E[(0, 0)][:, :], E[(0, 1)][:, :])
        nc.vector.tensor_add(ST[:, :], E[(1, 0)][:, :], E[(1, 1)][:, :])
        nc.vector.reciprocal(REC[:, :], ST[:, :])
        nc.vector.tensor_mul(RATIO[:, :], SR[:, :], REC[:, :])

        tot = ps1.tile([1, 4], dt.float32)
        nc.tensor.matmul(tot[:1, :4], ones[:, :1], RATIO[:, :4],
                         start=True, stop=True)
        red = stat.tile([1, 1], dt.float32, tag="red")
        nc.vector.tensor_reduce(red[:1, :1], tot[:1, :],
                                axis=mybir.AxisListType.X,
                                op=mybir.AluOpType.add)
        nc.sync.dma_start(out_d[:, :], red[:1, :1])

    nc.compile()
    return nc


def _build_w():
    """Unwindowed half-frame DFT weights + tridiagonal combine matrices."""
    theta = 2.0 * np.pi / 4096.0
    k = np.arange(2048, dtype=np.float64)

    def pack(fn):
        out = np.zeros((4, 128, 16, 128), np.float64)
        for c in range(4):
            nin = CHUNKS[c] + 2
            bins = np.arange(B0S[c] - 1, B0S[c] - 1 + nin, dtype=np.float64)
            W = fn(theta * np.outer(k, bins))       # [2048, nin]
            out[c, :, :, 0:nin] = W.reshape(16, 128, nin).transpose(1, 0, 2)
        return out.astype(FP8)

    # T tile: t[p, 0, j] = T1[j, p], t[p, 1, j] = T2[j, p] (tridiagonal,
    # translation-invariant, so one tile serves all chunks; all b0 are odd
    # so the (-1)^n column signs are identical across chunks).
    t = np.zeros((128, 2, 128), np.float64)
    for j in range(121):
        s = -1.0 if (j % 2 == 0) else 1.0   # (-1)^(b0+j), b0 odd
        t[j, 0, j] = -0.5
        t[j + 1, 0, j] = 1.0
        t[j + 2, 0, j] = -0.5
        t[j, 1, j] = 0.5 * s
        t[j + 1, 1, j] = 1.0 * s
        t[j + 2, 1, j] = 0.5 * s
    return {
        "wc": pack(np.cos),
        "ws": pack(np.sin),
        "tmat": t.astype(FP8),
    }


_CACHE: dict = {}


def _get_prog():
    if "nc" not in _CACHE:
        _CACHE["nc"] = _build_nc()
    return _CACHE["nc"]


def _get_w():
    if "w" not in _CACHE:
        _CACHE["w"] = _build_w()
    return _CACHE["w"]


def kernel(pred: np.ndarray, target: np.ndarray, _trace: bool = False):
    nc = _get_prog()
    w = _get_w()
    pred = np.asarray(pred)
    target = np.asarray(target)
    # only Welch rows 8..15 contribute -> only the right half of the columns
    tgt_half = np.ascontiguousarray(target[:, 8192:]).astype(np.float32)
    res_half = tgt_half - pred[:, 8192:]
    res8 = (res_half * XSCALE).astype(FP8)
    tgt8 = (tgt_half * XSCALE).astype(FP8)
    in_maps = []
    for i in range(N_CORES):
        sl = slice(1024 * i, 1024 * (i + 1))
        in_maps.append({
            "xr": np.ascontiguousarray(
                res8[:, sl].reshape(1024, 8, 128).transpose(2, 1, 0)),
            "xt": np.ascontiguousarray(
                tgt8[:, sl].reshape(1024, 8, 128).transpose(2, 1, 0)),
            **w,
        })
    res = run_bass_kernel_spmd(nc, in_maps, list(range(N_CORES)), trace=_trace)
    total = float(sum(float(res.results[i]["out"][0, 0])
                      for i in range(N_CORES)))
    out = np.array(total, dtype=np.float32)
    if _trace:
        return out, res
    return out


# revision 9
# speedup vs baseline: 3.4829x; 1.0093x over previous
"""CrossPSDLoss Trainium2 kernel (half-frame fp8 DoubleRow version).

Math (from the reference):
  res = target - pred; both [1024, 16384] f32.
  cross rows i=0..15: row i = concat_b x[b, 1024*i : 1024*(i+1)]  (length 1048576)
  Welch per row: 511 frames of 4096 (stride 2048), periodic-hann*2 window,
  rFFT, power, sum over frames -> S[n].  Loss only uses rows 8..15 and
  frequency bins 21..499, and the /T factors cancel in the ratio:
     out = (2/480) * sum_{row=8..15} sum_{n=21..499} S_res[row,n]/S_tgt[row,n]

Half-frame trick: the scaled periodic hann is win[k] = 1 - cos(theta k)
(theta = 2pi/4096), and win[k+2048] = 1 + cos(theta k).  With the
unwindowed half-frame DFT R_h[n] = sum_{k<2048} x[2048h+k] e^{-i theta n k}:
     F_f[n] = (R_f[n] - C_f[n]) + (-1)^n (R_{f+1}[n] + C_{f+1}[n]),
     C_h[n] = (R_h[n-1] + R_h[n+1]) / 2.
Each sample enters exactly one GEMM per (trig, bin-chunk) — the 50% frame
overlap is never recomputed, halving the main GEMM work.

Sharding: one Welch row per NeuronCore (8 rows, 8 cores); host sums the 8
per-core scalars.  Host computes res, scales by 1/4 (keeps R in fp8e4m3
range; the ratio is scale-invariant), casts to fp8, and pre-transposes to
XT[p, t, b] = X[b, 128t + p] so all DMAs are contiguous.

Per-core pipeline, per (input, trig, chunk-of-122-bins-with-halo) group:
  1. PE: 8 fp8 DoubleRow matmuls (K=256 each, 0.5 cycles/row in the cost
     model) -> psR[nin, 512] = R at the chunk's halo bin range x 512 halves.
  2. DVE: copy psR -> Rb (fp8, SBUF).
  3. PE: ONE DoubleRow matmul with a hand-built overlapping moving AP
     ([nin, 2@stride1, 511]) and the stacked tridiagonal weights
     T1 = tridiag(-1/2, 1, -1/2), T2 = diag((-1)^n) tridiag(1/2, 1, 1/2):
     psF = T1 @ Rb[:, 0:511] + T2 @ Rb[:, 1:512]  (both halves of F in one
     instruction, accumulated in PSUM; verified bit-exact on HW).
  4. ACT: Square activation with accum over the 511 frames -> E column.
PE emission is software-pipelined (group g's tridiag is emitted after
group g+1's main matmuls) so the PE never waits for the DVE bounce.
"""

import os
import sys
from contextlib import ExitStack

import numpy as np
import ml_dtypes

for _p in ("/opt/trn_rl_repo", "/root/.axon_site/_ro/trn_rl_repo"):
    if os.path.isdir(_p) and _p not in sys.path:
        sys.path.insert(0, _p)

import concourse.bass as bass
import concourse.mybir as mybir
from concourse import bacc, tile
from concourse.bass_utils import run_bass_kernel_spmd

FP8 = ml_dtypes.float8_e4m3

NSEG = 511
NH = 512             # half-frames
NBINS = 479          # bins 21..499
B0S = [21, 141, 261, 381]
CHUNKS = [120, 120, 120, 119]
N_CORES = 8
ROW0 = 8
XSCALE = 0.25


def _overlap_ap(ap2d):
    """[P, 512] 2D slice -> [P, 2, 511] AP, dim1 stride 1 (overlapping):
    element (p, r, f) reads column f + r."""
    y = ap2d.unsqueeze(1).copy()
    v = y.ap
    v.pop(1)
    v.insert(1, (1, 2))
    return y


def _build_nc() -> bass.Bass:
    nc = bacc.Bacc("TRN2", target_bir_lowering=False, debug=False,
                   num_devices=N_CORES)
    dt = mybir.dt
    DR = mybir.MatmulPerfMode.DoubleRow

    xr_d = nc.dram_tensor("xr", [128, 8, 1024], dt.float8e4,
                          kind="ExternalInput")
    xt_d = nc.dram_tensor("xt", [128, 8, 1024], dt.float8e4,
                          kind="ExternalInput")
    # half-DFT weights, chunk-major [chunk, p, m, 128]; cols j < nin are
    # bins b0-1+j (halo included), the rest zero padding (never read).
    wc_d = nc.dram_tensor("wc", [4, 128, 16, 128], dt.float8e4,
                          kind="ExternalInput")
    ws_d = nc.dram_tensor("ws", [4, 128, 16, 128], dt.float8e4,
                          kind="ExternalInput")
    t_d = nc.dram_tensor("tmat", [128, 2, 128], dt.float8e4,
                         kind="ExternalInput")
    out_d = nc.dram_tensor("out", [1, 1], dt.float32, kind="ExternalOutput")

    with ExitStack() as ctx:
        tc = ctx.enter_context(tile.TileContext(nc))
        xpool = ctx.enter_context(tc.tile_pool(name="x", bufs=1))
        wpool = ctx.enter_context(tc.tile_pool(name="w", bufs=1))
        psR = ctx.enter_context(tc.tile_pool(name="psR", bufs=3,
                                             space="PSUM"))
        psF = ctx.enter_context(tc.tile_pool(name="psF", bufs=3,
                                             space="PSUM"))
        ps1 = ctx.enter_context(tc.tile_pool(name="ps1", bufs=1, space="PSUM"))
        rbpool = ctx.enter_context(tc.tile_pool(name="rb", bufs=3))
        scpool = ctx.enter_context(tc.tile_pool(name="sc", bufs=2))
        stat = ctx.enter_context(tc.tile_pool(name="stat", bufs=1))

        xr_t = xpool.tile([128, 8, 1024], dt.float8e4, tag="xr")
        xt_t = xpool.tile([128, 8, 1024], dt.float8e4, tag="xt")
        t_t = wpool.tile([128, 2, 128], dt.float8e4, tag="tmat")
        w_sb = {}
        for trig in range(2):
            for c in range(4):
                w_sb[(trig, c)] = wpool.tile([128, 16, 128], dt.float8e4,
                                             name=f"w{trig}_{c}",
                                             tag=f"w{trig}_{c}")

        # E accumulators, column c = chunk c.  Partitions with no real bin
        # keep their memset value: res-E 0.0 / tgt-E 1.0 makes their ratio
        # an exact 0/2 = 0, so no masking is needed before the reduction.
        E = {}
        for xi in range(2):
            for trig in range(2):
                t_ = stat.tile([128, 4], dt.float32, name=f"E{xi}_{trig}",
                               tag=f"E{xi}_{trig}")
                nc.vector.memset(t_[:, :], 0.0 if xi == 0 else 1.0)
                E[(xi, trig)] = t_
        # the 2/480 loss scale rides on the reduction weights
        ones = stat.tile([128, 1], dt.float32, tag="ones")
        nc.vector.memset(ones[:, :], 2.0 / 480.0)

        # DMA order = PE need order (res groups run before tgt groups).
        nc.sync.dma_start(xr_t[:, 0:2, :], xr_d[:, 0:2, :])
        nc.sync.dma_start(w_sb[(0, 0)][:, :, :], wc_d[0])
        nc.sync.dma_start(xr_t[:, 2:4, :], xr_d[:, 2:4, :])
        nc.sync.dma_start(xr_t[:, 4:8, :], xr_d[:, 4:8, :])
        nc.sync.dma_start(w_sb[(0, 1)][:, :, :], wc_d[1])
        nc.sync.dma_start(t_t[:, :, :], t_d[:, :, :])
        nc.sync.dma_start(w_sb[(0, 2)][:, :, :], wc_d[2])
        nc.sync.dma_start(w_sb[(0, 3)][:, :, :], wc_d[3])
        for c in range(4):
            nc.sync.dma_start(w_sb[(1, c)][:, :, :], ws_d[c])
        nc.sync.dma_start(xt_t[:, 0:4, :], xt_d[:, 0:4, :])
        nc.sync.dma_start(xt_t[:, 4:8, :], xt_d[:, 4:8, :])

        # one group = (input, trig, chunk); res first so xt can arrive late.
        groups = [(xi, trig, c)
                  for xi in range(2) for trig in range(2) for c in range(4)]

        def emit_main(g):
            xi, trig, c = g
            xtile = xr_t if xi == 0 else xt_t
            nin = CHUNKS[c] + 2
            ps = psR.tile([128, NH], dt.float32, tag="psR")
            # pair order: (a=0,t=0..3), (a=1,t=0..3), (a=0,t=4..7), ... so
            # the first 4 matmuls only need the first half of the x tile.
            order = [0, 1, 4, 5, 2, 3, 6, 7]
            for n_, i in enumerate(order):
                m = 2 * i
                a, t = divmod(m, 8)
                nc.tensor.matmul(
                    ps[:nin, :],
                    w_sb[(trig, c)][:, m:m + 2, 0:nin],
                    xtile[:, t:t + 2, a:a + 1023:2],
                    start=(n_ == 0),
                    stop=(n_ == 7),
                    perf_mode=DR,
                )
            return ps

        def emit_bounce(g, ps):
            xi, trig, c = g
            nin = CHUNKS[c] + 2
            rb = rbpool.tile([128, NH], dt.float8e4, tag="rb")
            nc.vector.tensor_copy(rb[:nin, :], ps[:nin, :])
            return rb

        def emit_tail(g, rb):
            xi, trig, c = g
            rows = CHUNKS[c]
            nin = rows + 2
            pf = psF.tile([128, NSEG], dt.float32, tag="psF")
            nc.tensor.matmul(
                pf[:rows, :],
                t_t[0:nin, :, 0:rows],
                _overlap_ap(rb[0:nin, 0:NSEG]),
                start=True, stop=True,
                perf_mode=DR,
            )
            tmp = scpool.tile([128, NSEG], dt.float32, tag="sq")
            nc.scalar.activation(
                out=tmp[:rows, :],
                in_=pf[:rows, :],
                func=mybir.ActivationFunctionType.Square,
                accum_out=E[(xi, trig)][:rows, c:c + 1],
            )

        # ratio = (res_cos^2+res_sin^2) / (tgt_cos^2+tgt_sin^2); the res sum
        # is emitted mid-stream and the tgt side is folded per-column so only
        # chunk 3's reciprocal chain trails the final square.
        SR = stat.tile([128, 4], dt.float32, tag="SR")
        ST = stat.tile([128, 4], dt.float32, tag="ST")
        REC = stat.tile([128, 4], dt.float32, tag="REC")
        RATIO = stat.tile([128, 4], dt.float32, tag="RATIO")

        # software pipeline: group g's tridiag+square are emitted after
        # group g+1's main matmuls, so the PE never waits on the DVE copy.
        pend = None
        for gi, g in enumerate(groups):
            ps = emit_main(g)
            rb = emit_bounce(g, ps)
            if pend is not None:
                emit_tail(*pend)
            pend = (g, rb)
            if gi == 10:
                # all res squares (g0..g7) have long completed by now
                nc.vector.tensor_add(SR[:, :], E[(0, 0)][:, :],
                                     E[(0, 1)][:, :])
        emit_tail(*pend)

        for c in range(4):
            cs = slice(c, c + 1)
            nc.vector.tensor_add(ST[:, cs], E[(1, 0)][:, cs], E[(1, 1)][:, cs])
            nc.vector.reciprocal(REC[:, cs], ST[:, cs])
            nc.vector.tensor_mul(RATIO[:, cs], SR[:, cs], REC[:, cs])

        tot = ps1.tile([1, 4], dt.float32)
        nc.tensor.matmul(tot[:1, :4], ones[:, :1], RATIO[:, :4],
                         start=True, stop=True)
        red = stat.tile([1, 1], dt.float32, tag="red")
        nc.vector.tensor_reduce(red[:1, :1], tot[:1, :],
                                axis=mybir.AxisListType.X,
                                op=mybir.AluOpType.add)
        nc.sync.dma_start(out_d[:, :], red[:1, :1])

    nc.compile()
    return nc


def _build_w():
    """Unwindowed half-frame DFT weights + tridiagonal combine matrices."""
    theta = 2.0 * np.pi / 4096.0
    k = np.arange(2048, dtype=np.float64)

    def pack(fn):
        out = np.zeros((4, 128, 16, 128), np.float64)
        for c in range(4):
            nin = CHUNKS[c] + 2
            bins = np.arange(B0S[c] - 1, B0S[c] - 1 + nin, dtype=np.float64)
            W = fn(theta * np.outer(k, bins))       # [2048, nin]
            out[c, :, :, 0:nin] = W.reshape(16, 128, nin).transpose(1, 0, 2)
        return out.astype(FP8)

    # T tile: t[p, 0, j] = T1[j, p], t[p, 1, j] = T2[j, p] (tridiagonal,
    # translation-invariant, so one tile serves all chunks; all b0 are odd
    # so the (-1)^n column signs are identical across chunks).
    t = np.zeros((128, 2, 128), np.float64)
    for j in range(121):
        s = -1.0 if (j % 2 == 0) else 1.0   # (-1)^(b0+j), b0 odd
        t[j, 0, j] = -0.5
        t[j + 1, 0, j] = 1.0
        t[j + 2, 0, j] = -0.5
        t[j, 1, j] = 0.5 * s
        t[j + 1, 1, j] = 1.0 * s
        t[j + 2, 1, j] = 0.5 * s
    return {
        "wc": pack(np.cos),
        "ws": pack(np.sin),
        "tmat": t.astype(FP8),
    }


_CACHE: dict = {}


def _get_prog():
    if "nc" not in _CACHE:
        _CACHE["nc"] = _build_nc()
    return _CACHE["nc"]


def _get_w():
    if "w" not in _CACHE:
        _CACHE["w"] = _build_w()
    return _CACHE["w"]


def kernel(pred: np.ndarray, target: np.ndarray, _trace: bool = False):
    nc = _get_prog()
    w = _get_w()
    pred = np.asarray(pred)
    target = np.asarray(target)
    # only Welch rows 8..15 contribute -> only the right half of the columns
    tgt_half = np.ascontiguousarray(target[:, 8192:]).astype(np.float32)
    res_half = tgt_half - pred[:, 8192:]
    res8 = (res_half * XSCALE).astype(FP8)
    tgt8 = (tgt_half * XSCALE).astype(FP8)
    in_maps = []
    for i in range(N_CORES):
        sl = slice(1024 * i, 1024 * (i + 1))
        in_maps.append({
            "xr": np.ascontiguousarray(
                res8[:, sl].reshape(1024, 8, 128).transpose(2, 1, 0)),
            "xt": np.ascontiguousarray(
                tgt8[:, sl].reshape(1024, 8, 128).transpose(2, 1, 0)),
            **w,
        })
    res = run_bass_kernel_spmd(nc, in_maps, list(range(N_CORES)), trace=_trace)
    total = float(sum(float(res.results[i]["out"][0, 0])
                      for i in range(N_CORES)))
    out = np.array(total, dtype=np.float32)
    if _trace:
        return out, res
    return out
